# revision 30
# baseline (speedup 1.0000x reference)
"""Trainium2 Bass kernel for ExpKernelModule (Hawkes positive-likelihood intensities).

out[b,i] = sum_{j<i} alpha[u,v]*beta[u,v]*exp(clip(-beta[u,v]*(t_i-t_j), -20, 0))
with u=ct[b,i], v=ct[b,j], alpha=softplus(log_alpha), beta=softplus(log_beta).

Device algorithm (one batch per core, data-parallel over B=8):
the exp argument  log(a*b) - beta*(t_i - t_j)  is a rank-64 bilinear form over
the (receiver, trigger) type one-hots:

  arg[i,j] = W1[v,i]*oh[v,j] + W2[v,i]*(t_j*oh[v,j])     (sum over v)
  W1[v,i] = C1[u_i,v] - B[u_i,v]*t_i,  W2[v,i] = B[u_i,v],  oh[v,j] = 1[ct_j==v]

Per 128-row tile, matmuls produce the full exp-argument block in PSUM; ScalarE
applies Exp with a fused accum_out row-sum. Row tile r only needs columns
[0, 128*(r+1)); the diagonal 128x128 block gets a -1e4 additive strict-lower
mask (VectorE) before Exp.

PE dtype: float16. Each fp32 operand is split into a hi/lo fp16 pair (22
effective mantissa bits); per-operand errors scale with term magnitude, and
large-magnitude args are exactly the dead ones (exp ~ 0). Three accumulating
K=64 matmuls per chunk cover all needed hi/lo cross products:
  g1: [W1h, W1l] x [oh, oh]
  g2: [W2h, W2l] x [th*oh, th*oh]
  g3: [W2h, W2l] x [tl*oh, tl*oh]
Adjacent row tiles are processed as a pair, with the even tile's matmuls in PE
row-group 0 and the odd tile's in row-group 64 (tile_position packing): the
two K=64 streams run concurrently in disjoint array halves and LDWEIGHTS
pull-ahead applies, recovering most of the fp32-PSUM 2 cyc/col retire cost.
All five operand tensors are shipped with both partition halves populated so
either row-group can read them.
Measured end-to-end error vs the fp32 reference: ~7e-6 absmax-relative.
Host prep is O(L*D) index gathers only.
"""

import numpy as np

B_, L, D, P = 8, 2048, 32, 128
NT = L // P  # row tiles per batch
MASK_NEG = -1.0e4
MMW = 512  # moving-operand width per matmul (ISA limit for fp32 PSUM out)
MM_DTYPE = "float16"  # fp16 pairs: ~7e-6 err; "bfloat16" pairs: ~4e-4 err

_cached = {}


def _build_nc():
    import concourse.bass as bass  # noqa: F401
    import concourse.tile as tile
    from concourse import bacc, mybir

    f32 = mybir.dt.float32
    f16 = getattr(mybir.dt, MM_DTYPE)

    nc = bacc.Bacc("TRN2", target_bir_lowering=False, debug=False, num_devices=8)
    # each (128, L): rows [X, Y, X, Y] so both PE row-groups can read [X, Y]
    wg1_d = nc.dram_tensor("wg1", (4 * D, L), f16, kind="ExternalInput").ap()
    wg2_d = nc.dram_tensor("wg2", (4 * D, L), f16, kind="ExternalInput").ap()
    rg1_d = nc.dram_tensor("rg1", (4 * D, L), f16, kind="ExternalInput").ap()
    rg2_d = nc.dram_tensor("rg2", (4 * D, L), f16, kind="ExternalInput").ap()
    rg3_d = nc.dram_tensor("rg3", (4 * D, L), f16, kind="ExternalInput").ap()
    m_d = nc.dram_tensor("m", (P, P), f32, kind="ExternalInput").ap()
    # out[p, r] = row-sum for global row i = 128*r + p; one contiguous DMA
    o_d = nc.dram_tensor("o", (P, NT), f32, kind="ExternalOutput").ap()

    with tile.TileContext(nc) as tc:
        with (
            tc.tile_pool(name="singles", bufs=1) as singles,
            tc.tile_pool(name="psum_v6", bufs=2, space="PSUM") as psum,
            tc.tile_pool(name="acc", bufs=4) as accp,
        ):
            # Interleave input DMAs in consumption order (512-col pieces),
            # spread across the two HWDGE queues (sync + scalar) for overlap.
            wg1_sb = singles.tile([4 * D, L], f16)
            wg2_sb = singles.tile([4 * D, L], f16)
            rg1_sb = singles.tile([4 * D, L], f16)
            rg2_sb = singles.tile([4 * D, L], f16)
            rg3_sb = singles.tile([4 * D, L], f16)
            m_sb = singles.tile([P, P], f32)
            for c0 in range(0, L, 512):
                sl = slice(c0, c0 + 512)
                nc.sync.dma_start(rg1_sb[:, sl], rg1_d[:, sl])
                nc.scalar.dma_start(wg1_sb[:, sl], wg1_d[:, sl])
                nc.sync.dma_start(rg2_sb[:, sl], rg2_d[:, sl])
                nc.scalar.dma_start(wg2_sb[:, sl], wg2_d[:, sl])
                nc.sync.dma_start(rg3_sb[:, sl], rg3_d[:, sl])
                if c0 == 0:
                    nc.scalar.dma_start(m_sb[:, :], m_d[:, :])

            acc = accp.tile([P, NT], f32)
            half = {0: slice(0, 2 * D), 1: slice(2 * D, 4 * D)}
            for pr in range(NT // 2):
                rts = (2 * pr, 2 * pr + 1)
                pts = []
                for k, rt in enumerate(rts):
                    pts.append(
                        psum.tile([P, L], f32, tag="pt", name=f"pt_{pr}_{k}")
                    )
                ncols = [P * (rt + 1) for rt in rts]
                wsl = [slice(rt * P, (rt + 1) * P) for rt in rts]
                # even tile in PE rows 0-63, odd tile in rows 64-127: the two
                # K=64 streams run concurrently in disjoint array halves.
                for g, (w_sb, r_sb, st, sp) in enumerate(
                    [
                        (wg1_sb, rg1_sb, True, False),
                        (wg2_sb, rg2_sb, False, False),
                        (wg2_sb, rg3_sb, False, True),
                    ]
                ):
                    for c0 in range(0, ncols[1], MMW):
                        for k in (0, 1):
                            w_len = min(MMW, ncols[k] - c0)
                            if w_len <= 0:
                                continue
                            csl = slice(c0, c0 + w_len)
                            nc.tensor.matmul(
                                pts[k][:, csl],
                                w_sb[half[k], wsl[k]],
                                r_sb[half[k], csl],
                                start=st, stop=sp,
                            )
                for k, rt in enumerate(rts):
                    nck = ncols[k]
                    # strict-lower mask on the diagonal 128x128 block
                    nc.vector.tensor_add(
                        pts[k][:, nck - P : nck], pts[k][:, nck - P : nck], m_sb[:, :]
                    )
                    nc.scalar.activation(
                        pts[k][:, :nck],
                        pts[k][:, :nck],
                        mybir.ActivationFunctionType.Exp,
                        accum_out=acc[:, rt : rt + 1],
                    )
            nc.sync.dma_start(o_d[:, :], acc[:, :])

    nc.compile()
    return nc


def _softplus(x):
    return np.log1p(np.exp(-np.abs(x))) + np.maximum(x, 0.0)


def _host_prep(time_points, event_types, log_alpha, log_beta):
    t = np.asarray(time_points).astype(np.float64)  # (B, L)
    u = np.asarray(event_types).astype(np.int64)  # (B, L)
    A = _softplus(np.asarray(log_alpha).astype(np.float64))
    Bt = _softplus(np.asarray(log_beta).astype(np.float64))
    C1 = np.log(A * Bt)  # (D, D)

    if MM_DTYPE == "float16":
        f16 = np.float16
    else:
        import ml_dtypes

        f16 = ml_dtypes.bfloat16
    W1 = np.transpose(C1[u], (0, 2, 1)) - np.transpose(Bt[u], (0, 2, 1)) * t[:, None, :]
    W2 = np.transpose(Bt[u], (0, 2, 1))  # (B, D, L)
    W1h = W1.astype(f16); W1l = (W1 - W1h.astype(np.float64)).astype(f16)
    W2h = W2.astype(f16); W2l = (W2 - W2h.astype(np.float64)).astype(f16)
    th = t.astype(f16); tl = (t - th.astype(np.float64)).astype(f16)
    oh = (u[:, None, :] == np.arange(D)[None, :, None])  # (B, D, L) bool

    ohf = oh.astype(f16)
    tho = (th.astype(np.float64)[:, None, :] * oh).astype(f16)
    tlo = (tl.astype(np.float64)[:, None, :] * oh).astype(f16)
    WG1 = np.concatenate([W1h, W1l, W1h, W1l], axis=1)  # (B, 4D, L)
    WG2 = np.concatenate([W2h, W2l, W2h, W2l], axis=1)
    RG1 = np.concatenate([ohf, ohf, ohf, ohf], axis=1)
    RG2 = np.concatenate([tho, tho, tho, tho], axis=1)
    RG3 = np.concatenate([tlo, tlo, tlo, tlo], axis=1)
    mask = np.triu(np.full((P, P), MASK_NEG, dtype=np.float32), k=0)
    return WG1, WG2, RG1, RG2, RG3, mask


def _run(inputs, trace=False):
    from concourse.bass_utils import run_bass_kernel_spmd

    WG1, WG2, RG1, RG2, RG3, mask = _host_prep(
        inputs["time_points"],
        inputs["event_types"],
        inputs["log_alpha"],
        inputs["log_beta"],
    )
    if "nc" not in _cached:
        _cached["nc"] = _build_nc()
    nc = _cached["nc"]

    in_maps = [
        {"wg1": WG1[b], "wg2": WG2[b], "rg1": RG1[b], "rg2": RG2[b],
         "rg3": RG3[b], "m": mask}
        for b in range(B_)
    ]
    bres = run_bass_kernel_spmd(
        nc, in_maps, core_ids=list(range(B_)), trace=trace,
        trace_cores=[0] if trace else None,
    )
    # o is (P, NT) with out[i=128*r+p] = o[p, r]
    out = np.stack(
        [bres.results[b]["o"].reshape(P, NT).T.reshape(L) for b in range(B_)], axis=0
    )
    return out.astype(np.float32), bres


def kernel(**inputs) -> np.ndarray:
    out, _ = _run(inputs, trace=False)
    return out


# revision 31
# speedup vs baseline: 1.0828x; 1.0828x over previous
"""Trainium2 Bass kernel for ExpKernelModule (Hawkes positive-likelihood intensities).

out[b,i] = sum_{j<i} alpha[u,v]*beta[u,v]*exp(clip(-beta[u,v]*(t_i-t_j), -20, 0))
with u=ct[b,i], v=ct[b,j], alpha=softplus(log_alpha), beta=softplus(log_beta).

Device algorithm (one batch per core, data-parallel over B=8):
the exp argument  log(a*b) - beta*(t_i - t_j)  is a rank-64 bilinear form over
the (receiver, trigger) type one-hots:

  arg[i,j] = W1[v,i]*oh[v,j] + W2[v,i]*(t_j*oh[v,j])     (sum over v)
  W1[v,i] = C1[u_i,v] - B[u_i,v]*t_i,  W2[v,i] = B[u_i,v],  oh[v,j] = 1[ct_j==v]

Per 128-row tile, matmuls produce the full exp-argument block in PSUM; ScalarE
applies Exp with a fused accum_out row-sum. Row tile r only needs columns
[0, 128*(r+1)); the diagonal 128x128 block gets a -1e4 additive strict-lower
mask (VectorE) before Exp.

PE dtype: float16. Each fp32 operand is split into a hi/lo fp16 pair (22
effective mantissa bits); per-operand errors scale with term magnitude, and
large-magnitude args are exactly the dead ones (exp ~ 0). Two accumulating
matmuls per chunk cover all needed hi/lo cross products:
  mm1 K=128: [W1h, W1l, W2h, W2l] x [oh, oh, th*oh, th*oh]
  mm2 K=64:  [W2h, W2l]           x [tl*oh, tl*oh]
(Measured on HW: fp32-PSUM-accumulating matmuls retire at ~2 cyc/col for
bf16/fp16 alike — N-column stream time dominates and K is nearly free, so
fp16 costs the same as bf16 here and keeps fp32-level accuracy. Splitting
into more, narrower-K matmuls with row-group tile_position packing overlaps
streams but loses to the extra per-MM overhead. fp32 is 4 cyc/col; fp32r is
a 12-bit-mantissa mode.)
Measured end-to-end error vs the fp32 reference: ~7e-6 absmax-relative.
Host prep is O(L*D) index gathers only.
"""

import numpy as np

B_, L, D, P = 8, 2048, 32, 128
NT = L // P  # row tiles per batch
MASK_NEG = -1.0e4
MMW = 512  # moving-operand width per matmul (ISA limit for fp32 PSUM out)
MM_DTYPE = "float16"  # fp16 pairs: ~7e-6 err; "bfloat16" pairs: ~4e-4 err

_cached = {}


def _build_nc():
    import concourse.bass as bass  # noqa: F401
    import concourse.tile as tile
    from concourse import bacc, mybir

    f32 = mybir.dt.float32
    f16 = getattr(mybir.dt, MM_DTYPE)

    nc = bacc.Bacc("TRN2", target_bir_lowering=False, debug=False, num_devices=8)
    wa_d = nc.dram_tensor("wa", (4 * D, L), f16, kind="ExternalInput").ap()
    ra_d = nc.dram_tensor("ra", (4 * D, L), f16, kind="ExternalInput").ap()
    wb_d = nc.dram_tensor("wb", (2 * D, L), f16, kind="ExternalInput").ap()
    rb_d = nc.dram_tensor("rb", (2 * D, L), f16, kind="ExternalInput").ap()
    m_d = nc.dram_tensor("m", (P, P), f32, kind="ExternalInput").ap()
    # out[p, r] = row-sum for global row i = 128*r + p; one contiguous DMA
    o_d = nc.dram_tensor("o", (P, NT), f32, kind="ExternalOutput").ap()

    with tile.TileContext(nc) as tc:
        with (
            tc.tile_pool(name="singles", bufs=1) as singles,
            tc.tile_pool(name="psum_v7", bufs=2, space="PSUM") as psum,
            tc.tile_pool(name="acc", bufs=4) as accp,
        ):
            # Interleave input DMAs in consumption order (512-col pieces),
            # spread across the two HWDGE queues (sync + scalar) for overlap.
            wa_sb = singles.tile([4 * D, L], f16)
            wb_sb = singles.tile([2 * D, L], f16)
            ra_sb = singles.tile([4 * D, L], f16)
            rb_sb = singles.tile([2 * D, L], f16)
            m_sb = singles.tile([P, P], f32)
            for c0 in range(0, L, 512):
                sl = slice(c0, c0 + 512)
                nc.sync.dma_start(ra_sb[:, sl], ra_d[:, sl])
                nc.scalar.dma_start(wa_sb[:, sl], wa_d[:, sl])
                nc.sync.dma_start(rb_sb[:, sl], rb_d[:, sl])
                nc.scalar.dma_start(wb_sb[:, sl], wb_d[:, sl])
                if c0 == 0:
                    nc.scalar.dma_start(m_sb[:, :], m_d[:, :])

            acc = accp.tile([P, NT], f32)
            acc2 = accp.tile([P, 2], f32)
            for rt in range(NT):
                ncols = P * (rt + 1)
                pt = psum.tile([P, L], f32)
                wsl = slice(rt * P, (rt + 1) * P)
                # all mm1 chunks first, then all mm2 chunks: consecutive PE
                # matmuls hit different PSUM banks, so fill overlaps drain
                # (same-bank accumulate pairs back-to-back serialize the PE).
                for c0 in range(0, ncols, MMW):
                    w_len = min(MMW, ncols - c0)
                    csl = slice(c0, c0 + w_len)
                    nc.tensor.matmul(
                        pt[:, csl], wa_sb[:, wsl], ra_sb[:, csl],
                        start=True, stop=False,
                    )
                for c0 in range(0, ncols, MMW):
                    w_len = min(MMW, ncols - c0)
                    csl = slice(c0, c0 + w_len)
                    nc.tensor.matmul(
                        pt[:, csl], wb_sb[:, wsl], rb_sb[:, csl],
                        start=False, stop=True,
                    )
                # strict-lower mask on the diagonal 128x128 block
                nc.vector.tensor_add(
                    pt[:, ncols - P : ncols], pt[:, ncols - P : ncols], m_sb[:, :]
                )
                if rt < NT - 1:
                    nc.scalar.activation(
                        pt[:, :ncols],
                        pt[:, :ncols],
                        mybir.ActivationFunctionType.Exp,
                        accum_out=acc[:, rt : rt + 1],
                    )
                else:
                    # split the last (widest) Exp in two so its first half
                    # overlaps the final matmuls and the tail ACT is shorter
                    h = ncols // 2
                    nc.scalar.activation(
                        pt[:, :h], pt[:, :h],
                        mybir.ActivationFunctionType.Exp,
                        accum_out=acc2[:, 0:1],
                    )
                    nc.scalar.activation(
                        pt[:, h:ncols], pt[:, h:ncols],
                        mybir.ActivationFunctionType.Exp,
                        accum_out=acc2[:, 1:2],
                    )
                    nc.vector.tensor_add(
                        acc[:, rt : rt + 1], acc2[:, 0:1], acc2[:, 1:2]
                    )
            nc.sync.dma_start(o_d[:, :], acc[:, :])

    nc.compile()
    return nc


def _softplus(x):
    return np.log1p(np.exp(-np.abs(x))) + np.maximum(x, 0.0)


def _host_prep(time_points, event_types, log_alpha, log_beta):
    t = np.asarray(time_points).astype(np.float64)  # (B, L)
    u = np.asarray(event_types).astype(np.int64)  # (B, L)
    A = _softplus(np.asarray(log_alpha).astype(np.float64))
    Bt = _softplus(np.asarray(log_beta).astype(np.float64))
    C1 = np.log(A * Bt)  # (D, D)

    if MM_DTYPE == "float16":
        f16 = np.float16
    else:
        import ml_dtypes

        f16 = ml_dtypes.bfloat16
    W1 = np.transpose(C1[u], (0, 2, 1)) - np.transpose(Bt[u], (0, 2, 1)) * t[:, None, :]
    W2 = np.transpose(Bt[u], (0, 2, 1))  # (B, D, L)
    W1h = W1.astype(f16); W1l = (W1 - W1h.astype(np.float64)).astype(f16)
    W2h = W2.astype(f16); W2l = (W2 - W2h.astype(np.float64)).astype(f16)
    th = t.astype(f16); tl = (t - th.astype(np.float64)).astype(f16)
    oh = (u[:, None, :] == np.arange(D)[None, :, None])  # (B, D, L) bool

    WA = np.concatenate([W1h, W1l, W2h, W2l], axis=1)  # (B, 4D, L) f16
    RA = np.concatenate(
        [oh, oh,
         th.astype(np.float64)[:, None, :] * oh,
         th.astype(np.float64)[:, None, :] * oh], axis=1
    ).astype(f16)  # (B, 4D, L)
    WB = np.concatenate([W2h, W2l], axis=1)  # (B, 2D, L)
    tlo = tl.astype(np.float64)[:, None, :] * oh
    RB = np.concatenate([tlo, tlo], axis=1).astype(f16)  # (B, 2D, L)
    mask = np.triu(np.full((P, P), MASK_NEG, dtype=np.float32), k=0)
    return WA, RA, WB, RB, mask


def _run(inputs, trace=False):
    from concourse.bass_utils import run_bass_kernel_spmd

    WA, RA, WB, RB, mask = _host_prep(
        inputs["time_points"],
        inputs["event_types"],
        inputs["log_alpha"],
        inputs["log_beta"],
    )
    if "nc" not in _cached:
        _cached["nc"] = _build_nc()
    nc = _cached["nc"]

    in_maps = [
        {"wa": WA[b], "ra": RA[b], "wb": WB[b], "rb": RB[b], "m": mask}
        for b in range(B_)
    ]
    bres = run_bass_kernel_spmd(
        nc, in_maps, core_ids=list(range(B_)), trace=trace,
        trace_cores=[0] if trace else None,
    )
    # o is (P, NT) with out[i=128*r+p] = o[p, r]
    out = np.stack(
        [bres.results[b]["o"].reshape(P, NT).T.reshape(L) for b in range(B_)], axis=0
    )
    return out.astype(np.float32), bres


def kernel(**inputs) -> np.ndarray:
    out, _ = _run(inputs, trace=False)
    return out


# revision 32
# speedup vs baseline: 1.1887x; 1.0978x over previous
"""Trainium2 Bass kernel for ExpKernelModule (Hawkes positive-likelihood intensities).

out[b,i] = sum_{j<i} alpha[u,v]*beta[u,v]*exp(clip(-beta[u,v]*(t_i-t_j), -20, 0))
with u=ct[b,i], v=ct[b,j], alpha=softplus(log_alpha), beta=softplus(log_beta).

Device algorithm (one batch per core, data-parallel over B=8):
the exp argument  log(a*b) - beta*(t_i - t_j)  is a rank-64 bilinear form over
the (receiver, trigger) type one-hots:

  arg[i,j] = W1[v,i]*oh[v,j] + W2[v,i]*(t_j*oh[v,j])     (sum over v)
  W1[v,i] = C1[u_i,v] - B[u_i,v]*t_i,  W2[v,i] = B[u_i,v],  oh[v,j] = 1[ct_j==v]

Per 128-row tile, matmuls produce the full exp-argument block in PSUM; ScalarE
applies Exp with a fused accum_out row-sum. Row tile r only needs columns
[0, 128*(r+1)); the diagonal 128x128 block gets a -1e4 additive strict-lower
mask (VectorE) before Exp.

PE dtype: float16. Each fp32 operand is split into a hi/lo fp16 pair (22
effective mantissa bits); per-operand errors scale with term magnitude, and
large-magnitude args are exactly the dead ones (exp ~ 0). Two accumulating
matmuls per chunk cover all needed hi/lo cross products:
  mm1 K=128: [W1h, W1l, W2h, W2l] x [oh, oh, th*oh, th*oh]
  mm2 K=64:  [W2h, W2l]           x [tl*oh, tl*oh]
(Measured on HW: fp32-PSUM-accumulating matmuls retire at ~2 cyc/col for
bf16/fp16 alike — N-column stream time dominates and K is nearly free, so
fp16 costs the same as bf16 here and keeps fp32-level accuracy. Splitting
into more, narrower-K matmuls with row-group tile_position packing overlaps
streams but loses to the extra per-MM overhead. fp32 is 4 cyc/col; fp32r is
a 12-bit-mantissa mode.)
Measured end-to-end error vs the fp32 reference: ~7e-6 absmax-relative.
Host prep is O(L*D) index gathers only.
"""

import numpy as np

B_, L, D, P = 8, 2048, 32, 128
NT = L // P  # row tiles per batch
MASK_NEG = -1.0e4
MMW = 512  # moving-operand width per matmul (ISA limit for fp32 PSUM out)
MM_DTYPE = "float16"  # fp16 pairs: ~7e-6 err; "bfloat16" pairs: ~4e-4 err

_cached = {}


def _build_nc():
    import concourse.bass as bass  # noqa: F401
    import concourse.tile as tile
    from concourse import bacc, mybir

    f32 = mybir.dt.float32
    f16 = getattr(mybir.dt, MM_DTYPE)

    nc = bacc.Bacc("TRN2", target_bir_lowering=False, debug=False, enable_asserts=False, num_devices=8)
    wa_d = nc.dram_tensor("wa", (4 * D, L), f16, kind="ExternalInput").ap()
    ra_d = nc.dram_tensor("ra", (4 * D, L), f16, kind="ExternalInput").ap()
    wb_d = nc.dram_tensor("wb", (2 * D, L), f16, kind="ExternalInput").ap()
    rb_d = nc.dram_tensor("rb", (2 * D, L), f16, kind="ExternalInput").ap()
    m_d = nc.dram_tensor("m", (P, P), f32, kind="ExternalInput").ap()
    # out[p, r] = row-sum for global row i = 128*r + p; one contiguous DMA
    o_d = nc.dram_tensor("o", (P, NT), f32, kind="ExternalOutput").ap()

    with tile.TileContext(nc) as tc:
        with (
            tc.tile_pool(name="singles", bufs=1) as singles,
            tc.tile_pool(name="psum_v7", bufs=2, space="PSUM") as psum,
            tc.tile_pool(name="acc", bufs=4) as accp,
        ):
            # Interleave input DMAs in consumption order (512-col pieces),
            # spread across the two HWDGE queues (sync + scalar) for overlap.
            wa_sb = singles.tile([4 * D, L], f16)
            wb_sb = singles.tile([2 * D, L], f16)
            ra_sb = singles.tile([4 * D, L], f16)
            rb_sb = singles.tile([2 * D, L], f16)
            m_sb = singles.tile([P, P], f32)
            for c0 in range(0, L, 512):
                sl = slice(c0, c0 + 512)
                nc.sync.dma_start(ra_sb[:, sl], ra_d[:, sl])
                nc.scalar.dma_start(wa_sb[:, sl], wa_d[:, sl])
                nc.sync.dma_start(rb_sb[:, sl], rb_d[:, sl])
                nc.scalar.dma_start(wb_sb[:, sl], wb_d[:, sl])
                if c0 == 0:
                    nc.scalar.dma_start(m_sb[:, :], m_d[:, :])

            acc = accp.tile([P, NT], f32)
            acc2 = accp.tile([P, 2], f32)
            for rt in range(NT):
                ncols = P * (rt + 1)
                pt = psum.tile([P, L], f32)
                wsl = slice(rt * P, (rt + 1) * P)
                # all mm1 chunks first, then all mm2 chunks: consecutive PE
                # matmuls hit different PSUM banks, so fill overlaps drain
                # (same-bank accumulate pairs back-to-back serialize the PE).
                for c0 in range(0, ncols, MMW):
                    w_len = min(MMW, ncols - c0)
                    csl = slice(c0, c0 + w_len)
                    nc.tensor.matmul(
                        pt[:, csl], wa_sb[:, wsl], ra_sb[:, csl],
                        start=True, stop=False,
                    )
                for c0 in range(0, ncols, MMW):
                    w_len = min(MMW, ncols - c0)
                    csl = slice(c0, c0 + w_len)
                    nc.tensor.matmul(
                        pt[:, csl], wb_sb[:, wsl], rb_sb[:, csl],
                        start=False, stop=True,
                    )
                # strict-lower mask on the diagonal 128x128 block
                nc.vector.tensor_add(
                    pt[:, ncols - P : ncols], pt[:, ncols - P : ncols], m_sb[:, :]
                )
                if rt < NT - 1:
                    nc.scalar.activation(
                        pt[:, :ncols],
                        pt[:, :ncols],
                        mybir.ActivationFunctionType.Exp,
                        accum_out=acc[:, rt : rt + 1],
                    )
                else:
                    # split the last (widest) Exp in two so its first half
                    # overlaps the final matmuls and the tail ACT is shorter
                    h = ncols // 2
                    nc.scalar.activation(
                        pt[:, :h], pt[:, :h],
                        mybir.ActivationFunctionType.Exp,
                        accum_out=acc2[:, 0:1],
                    )
                    nc.scalar.activation(
                        pt[:, h:ncols], pt[:, h:ncols],
                        mybir.ActivationFunctionType.Exp,
                        accum_out=acc2[:, 1:2],
                    )
                    nc.vector.tensor_add(
                        acc[:, rt : rt + 1], acc2[:, 0:1], acc2[:, 1:2]
                    )
            nc.sync.dma_start(o_d[:, :], acc[:, :])

    nc.compile()
    return nc


def _softplus(x):
    return np.log1p(np.exp(-np.abs(x))) + np.maximum(x, 0.0)


def _host_prep(time_points, event_types, log_alpha, log_beta):
    t = np.asarray(time_points).astype(np.float64)  # (B, L)
    u = np.asarray(event_types).astype(np.int64)  # (B, L)
    A = _softplus(np.asarray(log_alpha).astype(np.float64))
    Bt = _softplus(np.asarray(log_beta).astype(np.float64))
    C1 = np.log(A * Bt)  # (D, D)

    if MM_DTYPE == "float16":
        f16 = np.float16
    else:
        import ml_dtypes

        f16 = ml_dtypes.bfloat16
    W1 = np.transpose(C1[u], (0, 2, 1)) - np.transpose(Bt[u], (0, 2, 1)) * t[:, None, :]
    W2 = np.transpose(Bt[u], (0, 2, 1))  # (B, D, L)
    W1h = W1.astype(f16); W1l = (W1 - W1h.astype(np.float64)).astype(f16)
    W2h = W2.astype(f16); W2l = (W2 - W2h.astype(np.float64)).astype(f16)
    th = t.astype(f16); tl = (t - th.astype(np.float64)).astype(f16)
    oh = (u[:, None, :] == np.arange(D)[None, :, None])  # (B, D, L) bool

    WA = np.concatenate([W1h, W1l, W2h, W2l], axis=1)  # (B, 4D, L) f16
    RA = np.concatenate(
        [oh, oh,
         th.astype(np.float64)[:, None, :] * oh,
         th.astype(np.float64)[:, None, :] * oh], axis=1
    ).astype(f16)  # (B, 4D, L)
    WB = np.concatenate([W2h, W2l], axis=1)  # (B, 2D, L)
    tlo = tl.astype(np.float64)[:, None, :] * oh
    RB = np.concatenate([tlo, tlo], axis=1).astype(f16)  # (B, 2D, L)
    mask = np.triu(np.full((P, P), MASK_NEG, dtype=np.float32), k=0)
    return WA, RA, WB, RB, mask


def _run(inputs, trace=False):
    from concourse.bass_utils import run_bass_kernel_spmd

    WA, RA, WB, RB, mask = _host_prep(
        inputs["time_points"],
        inputs["event_types"],
        inputs["log_alpha"],
        inputs["log_beta"],
    )
    if "nc" not in _cached:
        _cached["nc"] = _build_nc()
    nc = _cached["nc"]

    in_maps = [
        {"wa": WA[b], "ra": RA[b], "wb": WB[b], "rb": RB[b], "m": mask}
        for b in range(B_)
    ]
    bres = run_bass_kernel_spmd(
        nc, in_maps, core_ids=list(range(B_)), trace=trace,
        trace_cores=[0] if trace else None,
    )
    # o is (P, NT) with out[i=128*r+p] = o[p, r]
    out = np.stack(
        [bres.results[b]["o"].reshape(P, NT).T.reshape(L) for b in range(B_)], axis=0
    )
    return out.astype(np.float32), bres


def kernel(**inputs) -> np.ndarray:
    out, _ = _run(inputs, trace=False)
    return out
